# revision 15
# baseline (speedup 1.0000x reference)
"""Trainium2 Bass kernel for nn_DeconvolutionEnergyScoreLoss.

Self-contained: takes FULL inputs as in reference.setup_inputs(), shards
data-parallel over B=64 groups across 8 NeuronCores (8 groups/core),
runs one SPMD Bass program, gathers FULL outputs.

Returns (loss: np.float32, y_int: np.ndarray int32 [2048, 2000]).
"""

import numpy as np

import concourse.bacc as bacc
import concourse.bass as bass
import concourse.tile as tile
import concourse.bass_utils as bass_utils
from concourse import mybir

# problem constants (hardcoded per harness contract)
B, G, D, M, E, DIN = 64, 32, 2000, 16, 16, 256
LAM = 1.0
NCORES = 8
BL = B // NCORES            # 8 groups per core
TILES = 2                   # 2 y-tiles per core: [128 = 4b x 32g, D]
IPF_ITERS_DEV = 18          # converged to reference-60 fixed point by ~14
MCHUNKS = [(0, 512), (512, 512), (1024, 512), (1536, 464)]  # bank-aligned
PCH = [(0, 1024, [(0, 512), (512, 512)]),
       (1024, 976, [(0, 512), (512, 464)])]  # psum-chunk, width, sub-chunks
DCH = 16                    # d-chunks of 128 for transposes (last = 80)
F32 = mybir.dt.float32
BF16 = mybir.dt.bfloat16
I32 = mybir.dt.int32
AL = mybir.AluOpType
AF = mybir.ActivationFunctionType
AX = mybir.AxisListType

_PROG = None  # cached compiled program


def _dchunk(c):
    lo = c * 128
    return lo, min(128, D - lo)


def build_program(phases="ABCD"):
    import os
    phases = os.environ.get("KPHASES", phases)
    nc = bacc.Bacc("TRN2", target_bir_lowering=False, debug=False,
                   num_devices=NCORES)

    def din(name, shape, dt=F32):
        return nc.dram_tensor(name, list(shape), dt, kind="ExternalInput").ap()

    def dout(name, shape, dt=F32):
        return nc.dram_tensor(name, list(shape), dt, kind="ExternalOutput").ap()

    i_ls0 = din("ls0", [128, 256])        # sample lhsT rows 0..128   (x.T)
    i_ls1 = din("ls1", [128, 256])        # sample lhsT rows 128..256
    i_ls2 = din("ls2", [17, 256])         # [noise_sample.T ; ones]
    i_w0 = din("w0", [128, D])
    i_w1 = din("w1", [128, D])
    i_w2 = din("w2", [17, D])             # [Wn ; b]
    i_nlh = din("nlh", [128, 512])        # noise_loss [(8b,16m), (32g,16e)]
    i_trep = din("trep", [128, D])        # target replicated over m
    i_cf36 = din("cf36", [36, D])         # target_sum f32 rows 0-3 & 32-35
    i_l2i = din("l2i", [17, 128])         # loss lhsT chunk2 init (G row)
    i_cdt = din("cdt", [128, 128])        # C packed [(d%128), (16c, 8b)]
    i_ramp = din("ramp", [1, 4096], BF16)  # rank init: 31 - g
    i_eye = din("eye", [128, 128])        # identity (PE transpose)
    i_bdm = din("bdm", [128, 4])          # blockdiag mask p//32 == j
    i_bd16 = din("bd16", [128, 8])        # p//16 == j
    i_blk16 = din("blk16", [128, 128])    # p//16 == n//16
    i_map4 = din("map4", [36, 128])       # bcast lhsT rows {0-3,32-35}

    o_yint = dout("yint", [2 * 128, D], I32)
    o_loss = dout("loss8", [8, 1])

    import contextlib

    with tile.TileContext(nc) as tc, contextlib.ExitStack() as ctx:
        con = ctx.enter_context(tc.tile_pool(name="con", bufs=1))
        sb = ctx.enter_context(tc.tile_pool(name="sb", bufs=1))
        ps = ctx.enter_context(tc.tile_pool(name="ps", bufs=1, space="PSUM"))

        def load(pool, tag, ap_in, shape, dt=F32):
            t = pool.tile(shape, dt, tag=tag, name=tag)
            nc.sync.dma_start(out=t[:], in_=ap_in[:])
            return t

        ls0 = load(con, "ls0", i_ls0, [128, 256])
        ls1 = load(con, "ls1", i_ls1, [128, 256])
        ls2 = load(con, "ls2", i_ls2, [17, 256])
        # w tensors live in slots reused later by the rounding phase
        w0 = load(sb, "rA", i_w0, [128, D])
        w1 = load(sb, "rB", i_w1, [128, D])
        w2 = load(sb, "rC", i_w2, [17, D])
        nlh = load(con, "nlh", i_nlh, [128, 512])
        trep = load(sb, "trep", i_trep, [128, D])
        cf36 = load(con, "cf36", i_cf36, [36, D])
        cdt = load(con, "cdt", i_cdt, [128, 128])
        eye = load(con, "eye", i_eye, [128, 128])
        bdm = load(con, "bdm", i_bdm, [128, 4])
        bd16 = load(con, "bd16", i_bd16, [128, 8])
        blk16 = load(con, "blk16", i_blk16, [128, 128])
        map4 = load(con, "map4", i_map4, [36, 128])

        lsk = [ls0, ls1, ls2]
        wk = [w0, w1, w2]

        # =========== PHASE A: sample matmul + softplus -> y tiles ==========
        S = []            # y state tiles (post-cr), [128, D] each
        Rrow = []         # initial row sums R  [128, 1]
        for t in range(TILES):
            y = con.tile([128, D], F32, tag=f"y{t}", name=f"y{t}")
            r0 = con.tile([128, 1], F32, tag=f"r0{t}", name=f"r0{t}")
            r0p = sb.tile([128, 2], F32, tag="r0p", name=f"r0p{t}")
            for pi, (plo, pw, subs) in enumerate(PCH):
                pre = ps.tile([128, 1024], F32, tag="crb", bufs=2,
                              name=f"pre{t}_{pi}")
                for lo, wd in subs:
                    cs = slice(lo, lo + wd)
                    gs = slice(plo + lo, plo + lo + wd)
                    for k in range(3):
                        nc.tensor.matmul(
                            pre[:, cs], lsk[k][:, t * 128:(t + 1) * 128],
                            wk[k][:, gs], start=(k == 0), stop=(k == 2),
                        )
                gsl = slice(plo, plo + pw)
                ab = sb.tile([128, D], F32, tag="spA", name=f"ab{t}_{pi}")
                nc.scalar.activation(ab[:, 0:pw], pre[:, 0:pw], AF.Abs)
                ex = sb.tile([128, D], F32, tag="spB", name=f"ex{t}_{pi}")
                nc.scalar.activation(
                    ex[:, 0:pw], ab[:, 0:pw], AF.Exp, scale=-1.0)
                ln = sb.tile([128, D], F32, tag="scr", name=f"ln{t}_{pi}")
                nc.scalar.activation(
                    ln[:, 0:pw], ex[:, 0:pw], AF.Ln, bias=1.0)
                nc.vector.scalar_tensor_tensor(
                    out=y[:, gsl], in0=pre[:, 0:pw], scalar=0.0,
                    in1=ln[:, 0:pw],
                    op0=AL.max, op1=AL.add,
                    accum_out=r0p[:, pi:pi + 1],
                )
            nc.vector.tensor_tensor(
                r0[:], r0p[:, 0:1], r0p[:, 1:2], AL.add)
            S.append(y)
            Rrow.append(r0)

        # =========== PHASE D: loss path ==========
        run_d = "D" in phases
        lxs = []
        for t, lst in enumerate((ls0, ls1)):
            xsT = sb.tile([128, 8], F32, tag=f"xsT{t}", name=f"xsT{t}")
            nc.vector.tensor_reduce(
                xsT[:], lst[:].rearrange("p (b g) -> p b g", b=8),
                AX.X, AL.add,
            )
            lx = con.tile([128, 128], F32, tag=f"lx{t}", name=f"lx{t}")
            a = xsT[:]
            nc.vector.tensor_copy(
                lx[:].rearrange("p (b m) -> p b m", b=8),
                bass.AP(tensor=a.tensor, offset=a.offset,
                        ap=[list(a.ap[0]), [1, 8], [0, 16]]),
            )
            lxs.append(lx)
        ns2 = sb.tile([128, 16], F32, tag="ns2", name="ns2")
        nc.vector.tensor_reduce(
            ns2[:],
            nlh[:].rearrange("p (g e) -> p g e", g=32).transpose([0, 2, 1]),
            AX.X, AL.add,
        )
        nsT_ps = ps.tile([16, 128], F32, tag="col", bufs=2, name="nsT_ps")
        nc.tensor.transpose(nsT_ps[:], ns2[:], eye[:])
        lx2 = load(con, "lx2", i_l2i, [17, 128])
        nc.vector.tensor_copy(lx2[0:16, :], nsT_ps[:])
        lxs.append(lx2)

        ss = sb.tile([128, 1], F32, tag="ss", name="ss")
        sqp = sb.tile([128, 1], F32, tag="sqp", name="sqp")
        ssp = sb.tile([128, 2], F32, tag="ssp", name="ssp")
        sqpp = sb.tile([128, 2], F32, tag="sqpp", name="sqpp")
        pred_sb = sb.tile([128, D], F32, tag="spA", name="pred_sb")
        for pi, (plo, pw, subs) in enumerate(PCH):
            predp = ps.tile([128, 1024], F32, tag="crb", bufs=2,
                            name=f"predp{pi}")
            for lo, wd in subs:
                cs = slice(lo, lo + wd)
                gs = slice(plo + lo, plo + lo + wd)
                for k in range(3):
                    nc.tensor.matmul(
                        predp[:, cs], lxs[k][:], wk[k][:, gs],
                        start=(k == 0), stop=(k == 2),
                    )
            gsl = slice(plo, plo + pw)
            diff = sb.tile([128, D], F32, tag="scr", name=f"diff{pi}")
            nc.vector.tensor_tensor(
                diff[:, 0:pw], predp[:, 0:pw], trep[:, gsl], AL.subtract)
            dsq = sb.tile([128, D], F32, tag="dump", name=f"dsq{pi}")
            nc.scalar.activation(dsq[:, 0:pw], diff[:, 0:pw], AF.Square,
                                 accum_out=ssp[:, pi:pi + 1])
            psq = sb.tile([128, D], F32, tag="dump", name=f"psq{pi}")
            nc.scalar.activation(psq[:, 0:pw], predp[:, 0:pw], AF.Square,
                                 accum_out=sqpp[:, pi:pi + 1])
            nc.vector.tensor_copy(pred_sb[:, gsl], predp[:, 0:pw])
        nc.vector.tensor_tensor(ss[:], ssp[:, 0:1], ssp[:, 1:2], AL.add)
        nc.vector.tensor_tensor(sqp[:], sqpp[:, 0:1], sqpp[:, 1:2], AL.add)
        predT = sb.tile([128, 16 * 128], F32, tag="rA", name="predT")
        nc.vector.memset(predT[:], 0.0)
        for c in range(DCH):
            lo, w = _dchunk(c)
            tp = ps.tile([128, 128], F32, tag="col", bufs=2, name=f"tpl{c}")
            nc.tensor.transpose(tp[0:w, :], pred_sb[:, lo:lo + w], eye[:])
            nc.scalar.copy(predT[0:w, c * 128:(c + 1) * 128], tp[0:w, :])
        gram = ps.tile([128, 128], F32, tag="col", bufs=2, name="gram")
        for c in range(DCH):
            nc.tensor.matmul(
                gram[:], predT[:, c * 128:(c + 1) * 128],
                predT[:, c * 128:(c + 1) * 128],
                start=(c == 0), stop=(c == DCH - 1),
            )
        sqT_ps = ps.tile([1, 128], F32, tag="col", bufs=2, name="sqT_ps")
        nc.tensor.transpose(sqT_ps[:], sqp[:], eye[:])
        sqT = sb.tile([1, 128], F32, tag="sqT", name="sqT")
        nc.vector.tensor_copy(sqT[:], sqT_ps[:])
        ones1 = sb.tile([1, 128], F32, tag="ones1", name="ones1")
        nc.vector.memset(ones1[:], 1.0)
        sqrep = ps.tile([128, 128], F32, tag="col", bufs=2, name="sqrep")
        nc.tensor.matmul(sqrep[:], ones1[:], sqT[:], start=True, stop=True)
        sqrep_sb = sb.tile([128, 128], F32, tag="sqrep_sb", name="sqrep_sb")
        nc.vector.tensor_copy(sqrep_sb[:], sqrep[:])
        dmat = sb.tile([128, 128], F32, tag="dmat", name="dmat")
        nc.vector.scalar_tensor_tensor(
            out=dmat[:], in0=gram[:], scalar=-2.0, in1=sqrep_sb[:],
            op0=AL.mult, op1=AL.add,
        )
        nc.vector.tensor_scalar(dmat[:], dmat[:], sqp[:], 1e-6, AL.add, AL.max)
        nc.scalar.activation(dmat[:], dmat[:], AF.Sqrt)
        dblk = sb.tile([128, 128], F32, tag="dblk", name="dblk")
        nc.vector.tensor_tensor(dblk[:], dmat[:], blk16[:], AL.mult)
        rowpd = sb.tile([128, 1], F32, tag="rowpd", name="rowpd")
        nc.vector.tensor_reduce(rowpd[:], dblk[:], AX.X, AL.add)
        nc.vector.tensor_tensor(dblk[:], dmat[:], eye[:], AL.mult)
        ddiag = sb.tile([128, 1], F32, tag="ddiag", name="ddiag")
        nc.vector.tensor_reduce(ddiag[:], dblk[:], AX.X, AL.add)
        offd = sb.tile([128, 1], F32, tag="offd", name="offd")
        nc.vector.tensor_tensor(offd[:], rowpd[:], ddiag[:], AL.subtract)
        tcm = sb.tile([128, 1], F32, tag="tcm", name="tcm")
        nc.scalar.activation(tcm[:], ss[:], AF.Sqrt)
        pd8 = ps.tile([8, 1], F32, tag="col", bufs=2, name="pd8")
        nc.tensor.matmul(pd8[:], bd16[:], offd[:], start=True, stop=True)
        tc8 = ps.tile([8, 1], F32, tag="col", bufs=2, name="tc8")
        nc.tensor.matmul(tc8[:], bd16[:], tcm[:], start=True, stop=True)
        tc8s = sb.tile([8, 1], F32, tag="tc8s", name="tc8s")
        nc.vector.tensor_scalar_mul(tc8s[:], tc8[:], 1.0 / M)
        loss8 = sb.tile([8, 1], F32, tag="loss8", name="loss8")
        nc.vector.scalar_tensor_tensor(
            out=loss8[:], in0=pd8[:], scalar=-LAM / (2.0 * M * (M - 1)),
            in1=tc8s[:], op0=AL.mult, op1=AL.add,
        )
        nc.sync.dma_start(out=o_loss[:], in_=loss8[:])

        # =========== PHASE B: IPF ==========
        col36 = con.tile([36, D], F32, tag="col36", name="col36")
        nc.vector.memset(col36[:], 1.0)
        rr = []
        bd = []
        for t in range(TILES):
            r = con.tile([128, 1], F32, tag=f"rr{t}", name=f"rr{t}")
            nc.vector.memset(r[:], 1.0)
            rr.append(r)
            bdt = con.tile([128, 4], F32, tag=f"bd{t}", name=f"bd{t}")
            nc.vector.tensor_copy(bdt[:], bdm[:])
            bd.append(bdt)

        def ipf_round(i, clamp):
            for t in range(TILES):
                for plo, pw, subs in PCH:
                    colp = ps.tile([4, 1024], F32, tag="col", bufs=2,
                                   name=f"col{i}_{t}_{plo}")
                    for lo, wd in subs:
                        nc.tensor.matmul(
                            colp[:, lo:lo + wd], bd[t][:],
                            S[t][:, plo + lo:plo + lo + wd],
                            start=True, stop=True,
                        )
                    nc.scalar.copy(
                        col36[32 * t:32 * t + 4, plo:plo + pw],
                        colp[:, 0:pw])
            rec = sb.tile([36, D], F32, tag="spB", name=f"rec{i}")
            scr = sb.tile([36, D], F32, tag="scr", name=f"scr{i}")
            nc.vector.reciprocal_approx_accurate(
                out=rec[:], in_=col36[:], scratch=scr[:])
            crv = sb.tile([36, D], F32, tag="crv", name=f"crv{i}")
            nc.gpsimd.tensor_tensor(crv[:], rec[:], cf36[:], AL.mult)
            if clamp:
                nc.gpsimd.tensor_scalar(
                    crv[:], crv[:], 0.75, 1.25, AL.max, AL.min)
            rows = []
            for t in range(TILES):
                rowp = sb.tile([128, 2], F32, tag=f"rowp{t}",
                               name=f"rowp{i}_{t}")
                for pi, (plo, pw, subs) in enumerate(PCH):
                    crb = ps.tile([128, 1024], F32, tag="crb", bufs=2,
                                  name=f"crb{i}_{t}_{pi}")
                    for lo, wd in subs:
                        nc.tensor.matmul(
                            crb[:, lo:lo + wd], map4[32 * t:32 * t + 4, :],
                            crv[32 * t:32 * t + 4,
                                plo + lo:plo + lo + wd],
                            start=True, stop=True,
                        )
                    nc.vector.scalar_tensor_tensor(
                        out=S[t][:, plo:plo + pw], in0=S[t][:, plo:plo + pw],
                        scalar=rr[t][:], in1=crb[:, 0:pw],
                        op0=AL.mult, op1=AL.mult,
                        accum_out=rowp[:, pi:pi + 1],
                    )
                row = sb.tile([128, 1], F32, tag=f"row{t}", name=f"row{i}_{t}")
                nc.vector.tensor_tensor(
                    row[:], rowp[:, 0:1], rowp[:, 1:2], AL.add)
                rows.append(row)
            return rows

        for it in range(IPF_ITERS_DEV):
            rows = ipf_round(it, clamp=True)
            for t in range(TILES):
                rcp = sb.tile([128, 1], F32, tag=f"rcp{t}", name=f"rcp{it}_{t}")
                nc.vector.reciprocal(rcp[:], rows[t][:])
                rrn = sb.tile([128, 1], F32, tag=f"rrn{t}", name=f"rrn{it}_{t}")
                nc.vector.tensor_tensor(rrn[:], rcp[:], Rrow[t][:], AL.mult)
                nc.vector.tensor_scalar(
                    rr[t][:], rrn[:], 0.75, 1.25, AL.max, AL.min)
                a = rr[t][:]
                nc.vector.tensor_tensor(
                    bd[t][:],
                    bass.AP(tensor=a.tensor, offset=a.offset,
                            ap=[list(a.ap[0]), [0, 4]]),
                    bdm[:], AL.mult,
                )
        ipf_round(IPF_ITERS_DEV, clamp=False)   # final C/col scaling

        # =========== PHASE C: rounding ==========
        ydt = con.tile([128, 16, 8, 32], F32, tag="ydt", name="ydt")
        nc.vector.memset(ydt[:].rearrange("p c b g -> p (c b g)"), 0.0)
        for c in range(DCH):
            lo, w = _dchunk(c)
            for t in range(TILES):
                tp = ps.tile([128, 128], F32, tag="col", bufs=2, name=f"tpr{c}_{t}")
                nc.tensor.transpose(tp[0:w, :], S[t][:, lo:lo + w], eye[:])
                nc.scalar.copy(
                    ydt[0:w, c, 4 * t:4 * t + 4, :]
                    .rearrange("p b g -> p (b g)"),
                    tp[0:w, :],
                )
        ydtf = ydt[:].rearrange("p c b g -> p (c b g)")
        ti = sb.tile([128, 4096], I32, tag="rA", name="ti")
        nc.vector.tensor_copy(ti[:], ydtf)
        tf = sb.tile([128, 4096], F32, tag="rB", name="tf")
        nc.vector.tensor_copy(tf[:], ti[:])
        gtm = sb.tile([128, 4096], F32, tag="rA", name="gtm")
        nc.vector.tensor_tensor(gtm[:], tf[:], ydtf, AL.is_gt)
        fl = sb.tile([128, 4096], F32, tag="rC", name="fl")
        nc.vector.tensor_tensor(fl[:], tf[:], gtm[:], AL.subtract)
        frac = sb.tile([128, 16, 8, 32], F32, tag="rD", name="frac")
        fracf = frac[:].rearrange("p c b g -> p (c b g)")
        nc.vector.tensor_tensor(fracf, ydtf, fl[:], AL.subtract)
        colf = sb.tile([128, 128], F32, tag="colf", name="colf")
        nc.vector.tensor_reduce(
            colf[:], fl[:].rearrange("p (cb g) -> p cb g", g=32), AX.X, AL.add)
        need = sb.tile([128, 128], F32, tag="need", name="need")
        nc.vector.tensor_tensor(need[:], cdt[:], colf[:], AL.subtract)
        nc.vector.tensor_scalar_max(need[:], need[:], 0.0)
        qs = sb.tile([128, 128], F32, tag="qs", name="qs")
        nc.vector.tensor_scalar_mul(qs[:], need[:], 1.0 / G)
        qi = sb.tile([128, 128], I32, tag="qi", name="qi")
        nc.vector.tensor_copy(qi[:], qs[:])
        qf = sb.tile([128, 128], F32, tag="qf", name="qf")
        nc.vector.tensor_copy(qf[:], qi[:])
        qg = sb.tile([128, 128], F32, tag="qg", name="qg")
        nc.vector.tensor_tensor(qg[:], qf[:], qs[:], AL.is_gt)
        nc.vector.tensor_tensor(qf[:], qf[:], qg[:], AL.subtract)
        rsm = sb.tile([128, 128], F32, tag="rsm", name="rsm")
        nc.vector.scalar_tensor_tensor(
            out=rsm[:], in0=qf[:], scalar=-float(G), in1=need[:],
            op0=AL.mult, op1=AL.add,
        )
        # ranks
        sacc = sb.tile([128, 16, 8, 32], BF16, tag="sacc", name="sacc")
        nc.sync.dma_start(
            out=sacc[:].rearrange("p c b g -> p (c b g)"),
            in_=bass.AP(tensor=i_ramp.tensor, offset=i_ramp.offset,
                        ap=[[0, 128], [1, 4096]]),
        )
        cmp = sb.tile([128, 16, 8, 32], BF16, tag="cmp", name="cmp")
        for lag in range(1, 32):
            wl = 32 - lag
            nc.vector.tensor_tensor(
                cmp[:, :, :, 0:wl], frac[:, :, :, 0:wl],
                frac[:, :, :, lag:32], AL.is_ge)
            nc.vector.tensor_tensor(
                sacc[:, :, :, lag:32], sacc[:, :, :, lag:32],
                cmp[:, :, :, 0:wl], AL.add)
            nc.vector.tensor_tensor(
                sacc[:, :, :, 0:wl], sacc[:, :, :, 0:wl],
                cmp[:, :, :, 0:wl], AL.subtract)

        def pack_bcast(t128):
            a = t128[:]
            return bass.AP(tensor=a.tensor, offset=a.offset,
                           ap=[list(a.ap[0]), [8, 16], [1, 8], [0, 32]])

        rsmb = sb.tile([128, 128], BF16, tag="rsmb", name="rsmb")
        nc.vector.tensor_copy(rsmb[:], rsm[:])
        addm = sb.tile([128, 16, 8, 32], F32, tag="rB", name="addm")
        nc.vector.tensor_tensor(
            addm[:], sacc[:], pack_bcast(rsmb), AL.is_lt)
        yint = sb.tile([128, 16, 8, 32], F32, tag="rA", name="yint")
        nc.vector.tensor_tensor(
            yint[:], fl[:].rearrange("p (c b g) -> p c b g", c=16, b=8),
            pack_bcast(qf), AL.add)
        nc.vector.tensor_tensor(
            yint[:].rearrange("p c b g -> p (c b g)"),
            yint[:].rearrange("p c b g -> p (c b g)"),
            addm[:].rearrange("p c b g -> p (c b g)"), AL.add)
        for t in range(TILES):
            ob = sb.tile([128, D], I32, tag="ob", name=f"ob{t}")
            for c in range(DCH):
                lo, w = _dchunk(c)
                tp = ps.tile([128, 128], F32, tag="col", bufs=2, name=f"tpo{c}_{t}")
                nc.tensor.transpose(
                    tp[:, 0:w],
                    yint[0:w, c, 4 * t:4 * t + 4, :]
                    .rearrange("p b g -> p (b g)"),
                    eye[0:w, 0:w])
                nc.scalar.copy(ob[:, lo:lo + w], tp[:, 0:w])
            nc.sync.dma_start(
                out=o_yint[t * 128:(t + 1) * 128, :], in_=ob[:])

    nc.compile()
    return nc


def _prep_core_inputs(c, x, target, target_sum, W, b, noise_loss,
                      noise_sample):
    f32 = np.float32
    sl = slice(c * BL * G, (c + 1) * BL * G)
    xc = np.ascontiguousarray(x[sl]).astype(f32)                 # [256, 256]
    nsc = np.ascontiguousarray(noise_sample[sl]).astype(f32)     # [256, 16]
    nlc = np.ascontiguousarray(
        noise_loss[c * BL * G * M:(c + 1) * BL * G * M]).astype(f32)
    tc = np.ascontiguousarray(target[c * BL:(c + 1) * BL]).astype(f32)
    Cc = np.ascontiguousarray(target_sum[c * BL:(c + 1) * BL]).astype(
        np.int64)

    xT = xc.T.copy()
    ls0, ls1 = xT[0:128].copy(), xT[128:256].copy()
    ls2 = np.concatenate([nsc.T, np.ones((1, BL * G), f32)], axis=0)
    w0 = np.ascontiguousarray(W[0:128]).astype(f32)
    w1 = np.ascontiguousarray(W[128:256]).astype(f32)
    w2 = np.concatenate(
        [np.asarray(W[256:272], dtype=f32), np.asarray(b, f32)[None, :]], 0)
    nlh = (nlc.reshape(BL, G, M, E).transpose(0, 2, 1, 3)
           .reshape(128, G * E).astype(f32))
    trep = np.repeat(tc, M, axis=0)
    cf8 = Cc.astype(f32)
    cf36 = np.zeros((36, D), f32)
    cf36[0:4] = cf8[0:4]
    cf36[32:36] = cf8[4:8]
    l2i = np.zeros((17, 128), f32)
    l2i[16, :] = float(G)
    cdt = np.zeros((128, 128), f32)
    for ch in range(DCH):
        lo = ch * 128
        w = min(128, D - lo)
        cdt[0:w, ch * 8:(ch + 1) * 8] = cf8[:, lo:lo + w].T
    import ml_dtypes
    ramp = np.tile((31 - np.arange(32, dtype=np.int32)), 128)[None, :]
    ramp = ramp.astype(ml_dtypes.bfloat16)
    eye = np.eye(128, dtype=f32)
    bdm = np.zeros((128, 4), f32)
    for j in range(4):
        bdm[j * 32:(j + 1) * 32, j] = 1.0
    bd16 = np.zeros((128, 8), f32)
    for j in range(8):
        bd16[j * 16:(j + 1) * 16, j] = 1.0
    blk16 = (np.arange(128)[:, None] // 16 == np.arange(128)[None, :] // 16
             ).astype(f32)
    map4 = np.zeros((36, 128), f32)
    for p in range(128):
        map4[p // 32, p] = 1.0
        map4[32 + p // 32, p] = 1.0
    return {
        "ls0": ls0, "ls1": ls1, "ls2": ls2, "w0": w0, "w1": w1, "w2": w2,
        "nlh": nlh, "trep": trep, "cf36": cf36, "l2i": l2i,
        "cdt": cdt, "ramp": ramp,
        "eye": eye, "bdm": bdm, "bd16": bd16, "blk16": blk16,
        "map4": map4,
    }


def get_program():
    global _PROG
    if _PROG is None:
        _PROG = build_program()
    return _PROG


def run(x, target, target_sum, W, b, noise_loss, noise_sample, trace=False):
    nc = get_program()
    in_maps = [
        _prep_core_inputs(c, x, target, target_sum, W, b, noise_loss,
                          noise_sample)
        for c in range(NCORES)
    ]
    res = bass_utils.run_bass_kernel_spmd(
        nc, in_maps, core_ids=list(range(NCORES)), trace=trace)
    y_full = np.concatenate(
        [res.results[c]["yint"] for c in range(NCORES)], axis=0)
    loss_parts = np.concatenate(
        [res.results[c]["loss8"][:, 0] for c in range(NCORES)])
    loss = np.float32(np.mean(loss_parts.astype(np.float32)))
    return loss, y_full.astype(np.int32), res


def kernel(x, target, target_sum, W, b, noise_loss, noise_sample):
    loss, y_full, _ = run(x, target, target_sum, W, b, noise_loss,
                          noise_sample)
    return loss, y_full


# revision 16
# speedup vs baseline: 1.5972x; 1.5972x over previous
"""Trainium2 Bass kernel for nn_DeconvolutionEnergyScoreLoss.

Self-contained: takes FULL inputs as in reference.setup_inputs(), shards
data-parallel over B=64 groups across 8 NeuronCores (8 groups/core),
runs one SPMD Bass program, gathers FULL outputs.

Returns (loss: np.float32, y_int: np.ndarray int32 [2048, 2000]).
"""

import numpy as np

import concourse.bacc as bacc
import concourse.bass as bass
import concourse.tile as tile
import concourse.bass_utils as bass_utils
from concourse import mybir

# problem constants (hardcoded per harness contract)
B, G, D, M, E, DIN = 64, 32, 2000, 16, 16, 256
LAM = 1.0
NCORES = 8
BL = B // NCORES            # 8 groups per core
TILES = 2                   # 2 y-tiles per core: [128 = 4b x 32g, D]
IPF_ITERS_DEV = 18          # converged to reference-60 fixed point by ~14
MCHUNKS = [(0, 512), (512, 512), (1024, 512), (1536, 464)]  # bank-aligned
PCH = [(0, 1024, [(0, 512), (512, 512)]),
       (1024, 976, [(0, 512), (512, 464)])]  # psum-chunk, width, sub-chunks
DCH = 16                    # d-chunks of 128 for transposes (last = 80)
F32 = mybir.dt.float32
BF16 = mybir.dt.bfloat16
I32 = mybir.dt.int32
AL = mybir.AluOpType
AF = mybir.ActivationFunctionType
AX = mybir.AxisListType

_PROG = None  # cached compiled program


def _dchunk(c):
    lo = c * 128
    return lo, min(128, D - lo)


def build_program(phases="ABCD"):
    import os
    phases = os.environ.get("KPHASES", phases)
    nc = bacc.Bacc("TRN2", target_bir_lowering=False, debug=False,
                   num_devices=NCORES)

    def din(name, shape, dt=F32):
        return nc.dram_tensor(name, list(shape), dt, kind="ExternalInput").ap()

    def dout(name, shape, dt=F32):
        return nc.dram_tensor(name, list(shape), dt, kind="ExternalOutput").ap()

    i_ls0 = din("ls0", [128, 256])        # sample lhsT rows 0..128   (x.T)
    i_ls1 = din("ls1", [128, 256])        # sample lhsT rows 128..256
    i_ls2 = din("ls2", [17, 256])         # [noise_sample.T ; ones]
    i_w0 = din("w0", [128, D])
    i_w1 = din("w1", [128, D])
    i_w2 = din("w2", [17, D])             # [Wn ; b]
    i_nlh = din("nlh", [128, 512])        # noise_loss [(8b,16m), (32g,16e)]
    i_trep = din("trep", [128, D])        # target replicated over m
    i_cf36 = din("cf36", [36, D])         # target_sum f32 rows 0-3 & 32-35
    i_l2i = din("l2i", [17, 128])         # loss lhsT chunk2 init (G row)
    i_cdt = din("cdt", [128, 128])        # C packed [(d%128), (16c, 8b)]
    i_ramp = din("ramp", [1, 4096], BF16)  # rank init: 31 - g
    i_eye = din("eye", [128, 128])        # identity (PE transpose)
    i_bdm = din("bdm", [128, 4])          # blockdiag mask p//32 == j
    i_bd16 = din("bd16", [128, 8])        # p//16 == j
    i_blk16 = din("blk16", [128, 128])    # p//16 == n//16
    i_map4 = din("map4", [36, 128])       # bcast lhsT rows {0-3,32-35}

    o_yint = dout("yint", [2 * 128, D], I32)
    o_loss = dout("loss8", [8, 1])

    import contextlib

    with tile.TileContext(nc) as tc, contextlib.ExitStack() as ctx:
        con = ctx.enter_context(tc.tile_pool(name="con", bufs=1))
        sb = ctx.enter_context(tc.tile_pool(name="sb", bufs=1))
        ps = ctx.enter_context(tc.tile_pool(name="ps", bufs=1, space="PSUM"))

        def load(pool, tag, ap_in, shape, dt=F32):
            t = pool.tile(shape, dt, tag=tag, name=tag)
            nc.sync.dma_start(out=t[:], in_=ap_in[:])
            return t

        ls0 = load(con, "ls0", i_ls0, [128, 256])
        ls1 = load(con, "ls1", i_ls1, [128, 256])
        ls2 = load(con, "ls2", i_ls2, [17, 256])
        # w tensors live in slots reused later by the rounding phase
        w0 = load(sb, "rA", i_w0, [128, D])
        w1 = load(sb, "rB", i_w1, [128, D])
        w2 = load(sb, "rC", i_w2, [17, D])
        nlh = load(con, "nlh", i_nlh, [128, 512])
        trep = load(sb, "trep", i_trep, [128, D])
        cf36 = load(con, "cf36", i_cf36, [36, D])
        cdt = load(con, "cdt", i_cdt, [128, 128])
        eye = load(con, "eye", i_eye, [128, 128])
        bdm = load(con, "bdm", i_bdm, [128, 4])
        bd16 = load(con, "bd16", i_bd16, [128, 8])
        blk16 = load(con, "blk16", i_blk16, [128, 128])
        map4 = load(con, "map4", i_map4, [36, 128])

        lsk = [ls0, ls1, ls2]
        wk = [w0, w1, w2]

        # =========== PHASE A: sample matmul + softplus -> y tiles ==========
        S = []            # y state tiles (post-cr), [128, D] each
        Rrow = []         # initial row sums R  [128, 1]
        for t in range(TILES):
            y = con.tile([128, D], F32, tag=f"y{t}", name=f"y{t}")
            r0 = con.tile([128, 1], F32, tag=f"r0{t}", name=f"r0{t}")
            r0p = sb.tile([128, 2], F32, tag="r0p", name=f"r0p{t}")
            for pi, (plo, pw, subs) in enumerate(PCH):
                pre = ps.tile([128, 1024], F32, tag="crb", bufs=2,
                              name=f"pre{t}_{pi}")
                for lo, wd in subs:
                    cs = slice(lo, lo + wd)
                    gs = slice(plo + lo, plo + lo + wd)
                    for k in range(3):
                        nc.tensor.matmul(
                            pre[:, cs], lsk[k][:, t * 128:(t + 1) * 128],
                            wk[k][:, gs], start=(k == 0), stop=(k == 2),
                        )
                gsl = slice(plo, plo + pw)
                ab = sb.tile([128, D], F32, tag="spA", name=f"ab{t}_{pi}")
                nc.scalar.activation(ab[:, 0:pw], pre[:, 0:pw], AF.Abs)
                ex = sb.tile([128, D], F32, tag="spB", name=f"ex{t}_{pi}")
                nc.scalar.activation(
                    ex[:, 0:pw], ab[:, 0:pw], AF.Exp, scale=-1.0)
                ln = sb.tile([128, D], F32, tag="scr", name=f"ln{t}_{pi}")
                nc.scalar.activation(
                    ln[:, 0:pw], ex[:, 0:pw], AF.Ln, bias=1.0)
                nc.vector.scalar_tensor_tensor(
                    out=y[:, gsl], in0=pre[:, 0:pw], scalar=0.0,
                    in1=ln[:, 0:pw],
                    op0=AL.max, op1=AL.add,
                    accum_out=r0p[:, pi:pi + 1],
                )
            nc.vector.tensor_tensor(
                r0[:], r0p[:, 0:1], r0p[:, 1:2], AL.add)
            S.append(y)
            Rrow.append(r0)

        # =========== PHASE D: loss path ==========
        run_d = "D" in phases
        lxs = []
        for t, lst in enumerate((ls0, ls1)):
            xsT = sb.tile([128, 8], F32, tag=f"xsT{t}", name=f"xsT{t}")
            nc.vector.tensor_reduce(
                xsT[:], lst[:].rearrange("p (b g) -> p b g", b=8),
                AX.X, AL.add,
            )
            lx = con.tile([128, 128], F32, tag=f"lx{t}", name=f"lx{t}")
            a = xsT[:]
            nc.vector.tensor_copy(
                lx[:].rearrange("p (b m) -> p b m", b=8),
                bass.AP(tensor=a.tensor, offset=a.offset,
                        ap=[list(a.ap[0]), [1, 8], [0, 16]]),
            )
            lxs.append(lx)
        ns2 = sb.tile([128, 16], F32, tag="ns2", name="ns2")
        nc.vector.tensor_reduce(
            ns2[:],
            nlh[:].rearrange("p (g e) -> p g e", g=32).transpose([0, 2, 1]),
            AX.X, AL.add,
        )
        nsT_ps = ps.tile([16, 128], F32, tag="col", bufs=2, name="nsT_ps")
        nc.tensor.transpose(nsT_ps[:], ns2[:], eye[:])
        lx2 = load(con, "lx2", i_l2i, [17, 128])
        nc.vector.tensor_copy(lx2[0:16, :], nsT_ps[:])
        lxs.append(lx2)

        ss = sb.tile([128, 1], F32, tag="ss", name="ss")
        sqp = sb.tile([128, 1], F32, tag="sqp", name="sqp")
        ssp = sb.tile([128, 2], F32, tag="ssp", name="ssp")
        sqpp = sb.tile([128, 2], F32, tag="sqpp", name="sqpp")
        pred_sb = sb.tile([128, D], F32, tag="spA", name="pred_sb")
        for pi, (plo, pw, subs) in enumerate(PCH):
            predp = ps.tile([128, 1024], F32, tag="crb", bufs=2,
                            name=f"predp{pi}")
            for lo, wd in subs:
                cs = slice(lo, lo + wd)
                gs = slice(plo + lo, plo + lo + wd)
                for k in range(3):
                    nc.tensor.matmul(
                        predp[:, cs], lxs[k][:], wk[k][:, gs],
                        start=(k == 0), stop=(k == 2),
                    )
            gsl = slice(plo, plo + pw)
            diff = sb.tile([128, D], F32, tag="scr", name=f"diff{pi}")
            nc.vector.tensor_tensor(
                diff[:, 0:pw], predp[:, 0:pw], trep[:, gsl], AL.subtract)
            dsq = sb.tile([128, D], F32, tag="dump", name=f"dsq{pi}")
            nc.scalar.activation(dsq[:, 0:pw], diff[:, 0:pw], AF.Square,
                                 accum_out=ssp[:, pi:pi + 1])
            psq = sb.tile([128, D], F32, tag="dump", name=f"psq{pi}")
            nc.scalar.activation(psq[:, 0:pw], predp[:, 0:pw], AF.Square,
                                 accum_out=sqpp[:, pi:pi + 1])
            nc.vector.tensor_copy(pred_sb[:, gsl], predp[:, 0:pw])
        nc.vector.tensor_tensor(ss[:], ssp[:, 0:1], ssp[:, 1:2], AL.add)
        nc.vector.tensor_tensor(sqp[:], sqpp[:, 0:1], sqpp[:, 1:2], AL.add)
        predT = sb.tile([128, 16 * 128], F32, tag="rA", name="predT")
        nc.vector.memset(predT[:], 0.0)
        for c in range(DCH):
            lo, w = _dchunk(c)
            tp = ps.tile([128, 128], F32, tag="col", bufs=2, name=f"tpl{c}")
            nc.tensor.transpose(tp[0:w, :], pred_sb[:, lo:lo + w], eye[:])
            nc.scalar.copy(predT[0:w, c * 128:(c + 1) * 128], tp[0:w, :])
        gram = ps.tile([128, 128], F32, tag="col", bufs=2, name="gram")
        for c in range(DCH):
            nc.tensor.matmul(
                gram[:], predT[:, c * 128:(c + 1) * 128],
                predT[:, c * 128:(c + 1) * 128],
                start=(c == 0), stop=(c == DCH - 1),
            )
        sqT_ps = ps.tile([1, 128], F32, tag="col", bufs=2, name="sqT_ps")
        nc.tensor.transpose(sqT_ps[:], sqp[:], eye[:])
        sqT = sb.tile([1, 128], F32, tag="sqT", name="sqT")
        nc.vector.tensor_copy(sqT[:], sqT_ps[:])
        ones1 = sb.tile([1, 128], F32, tag="ones1", name="ones1")
        nc.vector.memset(ones1[:], 1.0)
        sqrep = ps.tile([128, 128], F32, tag="col", bufs=2, name="sqrep")
        nc.tensor.matmul(sqrep[:], ones1[:], sqT[:], start=True, stop=True)
        sqrep_sb = sb.tile([128, 128], F32, tag="sqrep_sb", name="sqrep_sb")
        nc.vector.tensor_copy(sqrep_sb[:], sqrep[:])
        dmat = sb.tile([128, 128], F32, tag="dmat", name="dmat")
        nc.vector.scalar_tensor_tensor(
            out=dmat[:], in0=gram[:], scalar=-2.0, in1=sqrep_sb[:],
            op0=AL.mult, op1=AL.add,
        )
        nc.vector.tensor_scalar(dmat[:], dmat[:], sqp[:], 1e-6, AL.add, AL.max)
        nc.scalar.activation(dmat[:], dmat[:], AF.Sqrt)
        dblk = sb.tile([128, 128], F32, tag="dblk", name="dblk")
        nc.vector.tensor_tensor(dblk[:], dmat[:], blk16[:], AL.mult)
        rowpd = sb.tile([128, 1], F32, tag="rowpd", name="rowpd")
        nc.vector.tensor_reduce(rowpd[:], dblk[:], AX.X, AL.add)
        nc.vector.tensor_tensor(dblk[:], dmat[:], eye[:], AL.mult)
        ddiag = sb.tile([128, 1], F32, tag="ddiag", name="ddiag")
        nc.vector.tensor_reduce(ddiag[:], dblk[:], AX.X, AL.add)
        offd = sb.tile([128, 1], F32, tag="offd", name="offd")
        nc.vector.tensor_tensor(offd[:], rowpd[:], ddiag[:], AL.subtract)
        tcm = sb.tile([128, 1], F32, tag="tcm", name="tcm")
        nc.scalar.activation(tcm[:], ss[:], AF.Sqrt)
        pd8 = ps.tile([8, 1], F32, tag="col", bufs=2, name="pd8")
        nc.tensor.matmul(pd8[:], bd16[:], offd[:], start=True, stop=True)
        tc8 = ps.tile([8, 1], F32, tag="col", bufs=2, name="tc8")
        nc.tensor.matmul(tc8[:], bd16[:], tcm[:], start=True, stop=True)
        tc8s = sb.tile([8, 1], F32, tag="tc8s", name="tc8s")
        nc.vector.tensor_scalar_mul(tc8s[:], tc8[:], 1.0 / M)
        loss8 = sb.tile([8, 1], F32, tag="loss8", name="loss8")
        nc.vector.scalar_tensor_tensor(
            out=loss8[:], in0=pd8[:], scalar=-LAM / (2.0 * M * (M - 1)),
            in1=tc8s[:], op0=AL.mult, op1=AL.add,
        )
        nc.sync.dma_start(out=o_loss[:], in_=loss8[:])

        # =========== PHASE B: IPF ==========
        col36 = con.tile([36, D], F32, tag="col36", name="col36")
        nc.vector.memset(col36[:], 1.0)
        rr = []
        bd = []
        for t in range(TILES):
            r = con.tile([128, 1], F32, tag=f"rr{t}", name=f"rr{t}")
            nc.vector.memset(r[:], 1.0)
            rr.append(r)
            bdt = con.tile([128, 4], F32, tag=f"bd{t}", name=f"bd{t}")
            nc.vector.tensor_copy(bdt[:], bdm[:])
            bd.append(bdt)

        def ipf_round(i, clamp):
            for t in range(TILES):
                for plo, pw, subs in PCH:
                    colp = ps.tile([4, 1024], F32, tag="col", bufs=2,
                                   name=f"col{i}_{t}_{plo}")
                    for lo, wd in subs:
                        nc.tensor.matmul(
                            colp[:, lo:lo + wd], bd[t][:],
                            S[t][:, plo + lo:plo + lo + wd],
                            start=True, stop=True,
                        )
                    nc.vector.tensor_copy(
                        col36[32 * t:32 * t + 4, plo:plo + pw],
                        colp[:, 0:pw])
            rec = sb.tile([36, D], F32, tag="spB", name=f"rec{i}")
            scr = sb.tile([36, D], F32, tag="scr", name=f"scr{i}")
            nc.vector.reciprocal_approx_accurate(
                out=rec[:], in_=col36[:], scratch=scr[:])
            crv = sb.tile([36, D], F32, tag="crv", name=f"crv{i}")
            nc.vector.tensor_tensor(crv[:], rec[:], cf36[:], AL.mult)
            if clamp:
                nc.vector.tensor_scalar(
                    crv[:], crv[:], 0.75, 1.25, AL.max, AL.min)
            rows = []
            for t in range(TILES):
                rowp = sb.tile([128, 2], F32, tag=f"rowp{t}",
                               name=f"rowp{i}_{t}")
                for pi, (plo, pw, subs) in enumerate(PCH):
                    crb = ps.tile([128, 1024], F32, tag="crb", bufs=2,
                                  name=f"crb{i}_{t}_{pi}")
                    for lo, wd in subs:
                        nc.tensor.matmul(
                            crb[:, lo:lo + wd], map4[32 * t:32 * t + 4, :],
                            crv[32 * t:32 * t + 4,
                                plo + lo:plo + lo + wd],
                            start=True, stop=True,
                        )
                    nc.vector.scalar_tensor_tensor(
                        out=S[t][:, plo:plo + pw], in0=S[t][:, plo:plo + pw],
                        scalar=rr[t][:], in1=crb[:, 0:pw],
                        op0=AL.mult, op1=AL.mult,
                        accum_out=rowp[:, pi:pi + 1],
                    )
                row = sb.tile([128, 1], F32, tag=f"row{t}", name=f"row{i}_{t}")
                nc.vector.tensor_tensor(
                    row[:], rowp[:, 0:1], rowp[:, 1:2], AL.add)
                rows.append(row)
            return rows

        for it in range(IPF_ITERS_DEV):
            rows = ipf_round(it, clamp=True)
            for t in range(TILES):
                rcp = sb.tile([128, 1], F32, tag=f"rcp{t}", name=f"rcp{it}_{t}")
                nc.vector.reciprocal(rcp[:], rows[t][:])
                rrn = sb.tile([128, 1], F32, tag=f"rrn{t}", name=f"rrn{it}_{t}")
                nc.vector.tensor_tensor(rrn[:], rcp[:], Rrow[t][:], AL.mult)
                nc.vector.tensor_scalar(
                    rr[t][:], rrn[:], 0.75, 1.25, AL.max, AL.min)
                a = rr[t][:]
                nc.vector.tensor_tensor(
                    bd[t][:],
                    bass.AP(tensor=a.tensor, offset=a.offset,
                            ap=[list(a.ap[0]), [0, 4]]),
                    bdm[:], AL.mult,
                )
        ipf_round(IPF_ITERS_DEV, clamp=False)   # final C/col scaling

        # =========== PHASE C: rounding ==========
        ydt = con.tile([128, 16, 8, 32], F32, tag="ydt", name="ydt")
        nc.vector.memset(ydt[:].rearrange("p c b g -> p (c b g)"), 0.0)
        for c in range(DCH):
            lo, w = _dchunk(c)
            for t in range(TILES):
                tp = ps.tile([128, 128], F32, tag="col", bufs=2, name=f"tpr{c}_{t}")
                nc.tensor.transpose(tp[0:w, :], S[t][:, lo:lo + w], eye[:])
                nc.scalar.copy(
                    ydt[0:w, c, 4 * t:4 * t + 4, :]
                    .rearrange("p b g -> p (b g)"),
                    tp[0:w, :],
                )
        ydtf = ydt[:].rearrange("p c b g -> p (c b g)")
        ti = sb.tile([128, 4096], I32, tag="rA", name="ti")
        nc.vector.tensor_copy(ti[:], ydtf)
        tf = sb.tile([128, 4096], F32, tag="rB", name="tf")
        nc.vector.tensor_copy(tf[:], ti[:])
        gtm = sb.tile([128, 4096], F32, tag="rA", name="gtm")
        nc.vector.tensor_tensor(gtm[:], tf[:], ydtf, AL.is_gt)
        fl = sb.tile([128, 4096], F32, tag="rC", name="fl")
        nc.vector.tensor_tensor(fl[:], tf[:], gtm[:], AL.subtract)
        frac = sb.tile([128, 16, 8, 32], F32, tag="rD", name="frac")
        fracf = frac[:].rearrange("p c b g -> p (c b g)")
        nc.vector.tensor_tensor(fracf, ydtf, fl[:], AL.subtract)
        colf = sb.tile([128, 128], F32, tag="colf", name="colf")
        nc.vector.tensor_reduce(
            colf[:], fl[:].rearrange("p (cb g) -> p cb g", g=32), AX.X, AL.add)
        need = sb.tile([128, 128], F32, tag="need", name="need")
        nc.vector.tensor_tensor(need[:], cdt[:], colf[:], AL.subtract)
        nc.vector.tensor_scalar_max(need[:], need[:], 0.0)
        qs = sb.tile([128, 128], F32, tag="qs", name="qs")
        nc.vector.tensor_scalar_mul(qs[:], need[:], 1.0 / G)
        qi = sb.tile([128, 128], I32, tag="qi", name="qi")
        nc.vector.tensor_copy(qi[:], qs[:])
        qf = sb.tile([128, 128], F32, tag="qf", name="qf")
        nc.vector.tensor_copy(qf[:], qi[:])
        qg = sb.tile([128, 128], F32, tag="qg", name="qg")
        nc.vector.tensor_tensor(qg[:], qf[:], qs[:], AL.is_gt)
        nc.vector.tensor_tensor(qf[:], qf[:], qg[:], AL.subtract)
        rsm = sb.tile([128, 128], F32, tag="rsm", name="rsm")
        nc.vector.scalar_tensor_tensor(
            out=rsm[:], in0=qf[:], scalar=-float(G), in1=need[:],
            op0=AL.mult, op1=AL.add,
        )
        # ranks
        sacc = sb.tile([128, 16, 8, 32], BF16, tag="sacc", name="sacc")
        nc.sync.dma_start(
            out=sacc[:].rearrange("p c b g -> p (c b g)"),
            in_=bass.AP(tensor=i_ramp.tensor, offset=i_ramp.offset,
                        ap=[[0, 128], [1, 4096]]),
        )
        cmp = sb.tile([128, 16, 8, 32], BF16, tag="cmp", name="cmp")
        for lag in range(1, 32):
            wl = 32 - lag
            nc.vector.tensor_tensor(
                cmp[:, :, :, 0:wl], frac[:, :, :, 0:wl],
                frac[:, :, :, lag:32], AL.is_ge)
            nc.vector.tensor_tensor(
                sacc[:, :, :, lag:32], sacc[:, :, :, lag:32],
                cmp[:, :, :, 0:wl], AL.add)
            nc.vector.tensor_tensor(
                sacc[:, :, :, 0:wl], sacc[:, :, :, 0:wl],
                cmp[:, :, :, 0:wl], AL.subtract)

        def pack_bcast(t128):
            a = t128[:]
            return bass.AP(tensor=a.tensor, offset=a.offset,
                           ap=[list(a.ap[0]), [8, 16], [1, 8], [0, 32]])

        rsmb = sb.tile([128, 128], BF16, tag="rsmb", name="rsmb")
        nc.vector.tensor_copy(rsmb[:], rsm[:])
        addm = sb.tile([128, 16, 8, 32], F32, tag="rB", name="addm")
        nc.vector.tensor_tensor(
            addm[:], sacc[:], pack_bcast(rsmb), AL.is_lt)
        yint = sb.tile([128, 16, 8, 32], F32, tag="rA", name="yint")
        nc.vector.tensor_tensor(
            yint[:], fl[:].rearrange("p (c b g) -> p c b g", c=16, b=8),
            pack_bcast(qf), AL.add)
        nc.vector.tensor_tensor(
            yint[:].rearrange("p c b g -> p (c b g)"),
            yint[:].rearrange("p c b g -> p (c b g)"),
            addm[:].rearrange("p c b g -> p (c b g)"), AL.add)
        for t in range(TILES):
            ob = sb.tile([128, D], I32, tag="ob", name=f"ob{t}")
            for c in range(DCH):
                lo, w = _dchunk(c)
                tp = ps.tile([128, 128], F32, tag="col", bufs=2, name=f"tpo{c}_{t}")
                nc.tensor.transpose(
                    tp[:, 0:w],
                    yint[0:w, c, 4 * t:4 * t + 4, :]
                    .rearrange("p b g -> p (b g)"),
                    eye[0:w, 0:w])
                nc.scalar.copy(ob[:, lo:lo + w], tp[:, 0:w])
            nc.sync.dma_start(
                out=o_yint[t * 128:(t + 1) * 128, :], in_=ob[:])

    nc.compile()
    return nc


def _prep_core_inputs(c, x, target, target_sum, W, b, noise_loss,
                      noise_sample):
    f32 = np.float32
    sl = slice(c * BL * G, (c + 1) * BL * G)
    xc = np.ascontiguousarray(x[sl]).astype(f32)                 # [256, 256]
    nsc = np.ascontiguousarray(noise_sample[sl]).astype(f32)     # [256, 16]
    nlc = np.ascontiguousarray(
        noise_loss[c * BL * G * M:(c + 1) * BL * G * M]).astype(f32)
    tc = np.ascontiguousarray(target[c * BL:(c + 1) * BL]).astype(f32)
    Cc = np.ascontiguousarray(target_sum[c * BL:(c + 1) * BL]).astype(
        np.int64)

    xT = xc.T.copy()
    ls0, ls1 = xT[0:128].copy(), xT[128:256].copy()
    ls2 = np.concatenate([nsc.T, np.ones((1, BL * G), f32)], axis=0)
    w0 = np.ascontiguousarray(W[0:128]).astype(f32)
    w1 = np.ascontiguousarray(W[128:256]).astype(f32)
    w2 = np.concatenate(
        [np.asarray(W[256:272], dtype=f32), np.asarray(b, f32)[None, :]], 0)
    nlh = (nlc.reshape(BL, G, M, E).transpose(0, 2, 1, 3)
           .reshape(128, G * E).astype(f32))
    trep = np.repeat(tc, M, axis=0)
    cf8 = Cc.astype(f32)
    cf36 = np.zeros((36, D), f32)
    cf36[0:4] = cf8[0:4]
    cf36[32:36] = cf8[4:8]
    l2i = np.zeros((17, 128), f32)
    l2i[16, :] = float(G)
    cdt = np.zeros((128, 128), f32)
    for ch in range(DCH):
        lo = ch * 128
        w = min(128, D - lo)
        cdt[0:w, ch * 8:(ch + 1) * 8] = cf8[:, lo:lo + w].T
    import ml_dtypes
    ramp = np.tile((31 - np.arange(32, dtype=np.int32)), 128)[None, :]
    ramp = ramp.astype(ml_dtypes.bfloat16)
    eye = np.eye(128, dtype=f32)
    bdm = np.zeros((128, 4), f32)
    for j in range(4):
        bdm[j * 32:(j + 1) * 32, j] = 1.0
    bd16 = np.zeros((128, 8), f32)
    for j in range(8):
        bd16[j * 16:(j + 1) * 16, j] = 1.0
    blk16 = (np.arange(128)[:, None] // 16 == np.arange(128)[None, :] // 16
             ).astype(f32)
    map4 = np.zeros((36, 128), f32)
    for p in range(128):
        map4[p // 32, p] = 1.0
        map4[32 + p // 32, p] = 1.0
    return {
        "ls0": ls0, "ls1": ls1, "ls2": ls2, "w0": w0, "w1": w1, "w2": w2,
        "nlh": nlh, "trep": trep, "cf36": cf36, "l2i": l2i,
        "cdt": cdt, "ramp": ramp,
        "eye": eye, "bdm": bdm, "bd16": bd16, "blk16": blk16,
        "map4": map4,
    }


def get_program():
    global _PROG
    if _PROG is None:
        _PROG = build_program()
    return _PROG


def run(x, target, target_sum, W, b, noise_loss, noise_sample, trace=False):
    nc = get_program()
    in_maps = [
        _prep_core_inputs(c, x, target, target_sum, W, b, noise_loss,
                          noise_sample)
        for c in range(NCORES)
    ]
    res = bass_utils.run_bass_kernel_spmd(
        nc, in_maps, core_ids=list(range(NCORES)), trace=trace)
    y_full = np.concatenate(
        [res.results[c]["yint"] for c in range(NCORES)], axis=0)
    loss_parts = np.concatenate(
        [res.results[c]["loss8"][:, 0] for c in range(NCORES)])
    loss = np.float32(np.mean(loss_parts.astype(np.float32)))
    return loss, y_full.astype(np.int32), res


def kernel(x, target, target_sum, W, b, noise_loss, noise_sample):
    loss, y_full, _ = run(x, target, target_sum, W, b, noise_loss,
                          noise_sample)
    return loss, y_full
